# revision 44
# baseline (speedup 1.0000x reference)
"""Trainium2 Bass kernel for nn_ExpertGroup (moe_routing).

Sharding: 8 cores = (batch b in 0..3) x (seq half j in 0..1); each core owns
1024 tokens. Activations flow in transposed [feature, token] layout so every
matmul contracts over the partition dim. The sequence-mixing adapt attention
needs full-S adapt_in/adapt_out, so paired cores AllGather their N-layout
halves, overlapped with independent work.

Key structure (v3):
- Wc = 0.1*(Wo@Wp).T and Wda = 0.1*(Wd@Wap).T are computed on the HOST.
  The adapt correction to `hidden` never touches Wd on device: the final
  output is tcomb + Wc.T@hwT + Wda.T@(adapt * sumw), so the big Wd
  down-projection (psh) runs immediately after up/gate instead of waiting
  for the collective + adapt attention.
- Engine queues execute in emission order, so the tail is hand-woven:
  psh groups (PE), collective-2 readback, aoutT transposes, and aw/silu/
  adapt steps interleave one t-step per psh group; the final combine
  (pct) streams right after adT.
- The expert branch's matmuls run early (only need preT); its LN math
  runs on the otherwise-idle Pool engine (batched stats across all 8
  experts), with the per-expert sum/sq reduces woven into the tail's
  DVE slack via side_q.
- All input DMAs are host-packed to partition-major contiguous layouts
  (2KB descriptors); collective payloads use the same layout. The
  collective-1 send is emitted after P3 so its DMAs never steal
  bandwidth from the critical wu/wg weight stream.
- Output is bf16 on device (host upcasts): halves the output-DMA tail;
  error contribution ~8e-4 against the 2e-2 budget.
"""

import numpy as np
import ml_dtypes

import concourse.bacc as bacc
import concourse.mybir as mybir
import concourse.tile as tile
from concourse import bass_utils

F32 = mybir.dt.float32
BF16 = mybir.dt.bfloat16
AX = mybir.AxisListType
OP = mybir.AluOpType
AF = mybir.ActivationFunctionType

B, S, D, H, AD, E = 4, 2048, 1024, 2048, 128, 8
PHASES = []


def _mark(nc, name):
    PHASES.append((name, nc.next_id()))

TOK = 1024          # tokens per core
N_CORES = 8
NCH = TOK // 512    # 512-wide matmul chunks of the own token range
BF = ml_dtypes.bfloat16

_NC_CACHE = None


def build(fake_cc=False, reps=1):
    nc = bacc.Bacc("TRN2", target_bir_lowering=False, debug=False,
                   num_devices=N_CORES)

    # ---- per-core DRAM parameters (all packed host-side for 2KB-descriptor
    # partition-major DMAs) ----
    P = {}
    def par(name, shape, dt=BF16, out=False):
        P[name] = nc.declare_dram_parameter(name, shape, dt, isOutput=out)

    par("xt", [128, 8, TOK])
    par("wu_t", [16, 128, 8, 128])
    par("wg_t", [16, 128, 8, 128])
    par("wd_t", [8, 128, 16, 128])
    par("wpre_p", [128, 8, AD])
    par("wpost_p", [128, 16, AD])
    par("wc", [AD, D])
    par("wda", [AD, D])
    par("at_p", [AD, E, AD])
    par("bias_pk", [128, 40], F32)      # cols: bu[16] | bg[16] | bd[8]
    par("smalls", [128, 4], F32)        # cols: bpre | bpost | ln_g | ln_b
    par("smalls_row", [1, 256], F32)    # ln_g | ln_b
    par("eg_row", [1, E * AD], F32)
    par("eb", [E, AD], F32)
    par("ew_p2", [128, E, 8], F32)
    par("ewt", [E, TOK], F32)
    par("id_bf", [128, 128])
    par("id_f32", [128, 128], F32)
    par("out", [D, TOK], BF16, out=True)

    with tile.TileContext(nc) as tc:
        for r in range(reps):
            _emit(nc, tc, P, fake_cc, pfx=(f"r{r}_" if reps > 1 else ""))
    nc.compile()
    return nc


def _emit(nc, tc, P, fake_cc=False, pfx=""):
    import contextlib
    stack = contextlib.ExitStack()
    pool = stack.enter_context(tc.tile_pool(name=pfx + "res", bufs=1))
    scr = stack.enter_context(tc.tile_pool(name=pfx + "scr", bufs=2))
    wpool = stack.enter_context(tc.tile_pool(name=pfx + "wts", bufs=2))
    ps = stack.enter_context(tc.tile_pool(name=pfx + "ps", bufs=2, space="PSUM"))
    dram = stack.enter_context(tc.tile_pool(name=pfx + "dram", bufs=1, space="DRAM"))

    # =================== P0: input loads ===================
    _mark(nc, "P0")
    # critical-path loads first (P1 needs wpre+xt)
    xt_sb = pool.tile([128, 8, TOK], BF16, tag="xt_sb")
    nc.sync.dma_start(xt_sb[:, :, 0:512], P["xt"].ap()[:, :, 0:512])
    wpre_sb = pool.tile([128, 8, AD], BF16, tag="wpre_sb")
    nc.scalar.dma_start(wpre_sb[:], P["wpre_p"][:])
    nc.scalar.dma_start(xt_sb[:, :, 512:1024], P["xt"].ap()[:, :, 512:1024])
    ident_b = pool.tile([128, 128], BF16, tag="ident_b")
    ident_f = pool.tile([128, 128], F32, tag="ident_f")
    nc.scalar.dma_start(ident_b[:], P["id_bf"][:])
    nc.scalar.dma_start(ident_f[:], P["id_f32"][:])

    biask = pool.tile([128, 40], F32, tag="biask")
    nc.scalar.dma_start(biask[:], P["bias_pk"][:])
    but = biask[:, 0:16]
    bgt = biask[:, 16:32]
    bdt = biask[:, 32:40]
    smalls_sb = pool.tile([128, 4], F32, tag="smalls_sb")
    nc.scalar.dma_start(smalls_sb[:], P["smalls"][:])
    bpre_c = smalls_sb[:, 0:1]
    bpost_c = smalls_sb[:, 1:2]
    lng_c = smalls_sb[:, 2:3]
    lnb_c = smalls_sb[:, 3:4]
    srow = pool.tile([1, 256], F32, tag="srow")
    nc.scalar.dma_start(srow[:], P["smalls_row"][:])
    gB = pool.tile([128, 128], F32, tag="gB")
    bB = pool.tile([128, 128], F32, tag="bB")
    nc.gpsimd.partition_broadcast(gB[:], srow[:, 0:128])
    nc.gpsimd.partition_broadcast(bB[:], srow[:, 128:256])

    at_sb = pool.tile([128, E, AD], BF16, tag="at_sb")
    nc.gpsimd.dma_start(at_sb[:], P["at_p"][:])
    wpost_sb = pool.tile([128, 16, AD], BF16, tag="wpost_sb")
    nc.gpsimd.dma_start(wpost_sb[:], P["wpost_p"][:])

    # ---- LN helper (N-layout [128 tok, nb, 128 ad] blocks) ----
    def ln_stats(src, nb, tag):
        red = scr.tile([128, nb], F32, tag=tag + "_red")
        nc.vector.tensor_reduce(red[:], src[:], AX.X, OP.add)
        sq = scr.tile([128, nb, 128], BF16, tag=tag + "_sq", bufs=1)
        nc.gpsimd.tensor_tensor(sq[:], src[:], src[:], OP.mult)
        red2 = scr.tile([128, nb], F32, tag=tag + "_red2")
        nc.vector.tensor_reduce(red2[:], sq[:], AX.X, OP.add)
        t = scr.tile([128, nb], F32, tag=tag + "_t")
        nc.vector.tensor_tensor(t[:], red[:], red[:], OP.mult)
        v2 = scr.tile([128, nb], F32, tag=tag + "_v2")
        nc.vector.scalar_tensor_tensor(v2[:], t[:], -1.0 / AD, red2[:],
                                       OP.mult, OP.add)
        v3 = scr.tile([128, nb], F32, tag=tag + "_v3")
        nc.vector.tensor_scalar(v3[:], v2[:], 1.0 / AD, 1e-5, OP.mult, OP.add)
        sd = scr.tile([128, nb], F32, tag=tag + "_sd")
        nc.scalar.activation(sd[:], v3[:], AF.Sqrt)
        rs = scr.tile([128, nb], F32, tag=tag + "_rs")
        nc.vector.reciprocal(rs[:], sd[:])
        m = scr.tile([128, nb], F32, tag=tag + "_m")
        nc.vector.tensor_scalar_mul(m[:], red[:], 1.0 / AD)
        return m, rs

    def layer_norm(src, nb, dst, tag, apply_gb=True):
        m, rs = ln_stats(src, nb, tag)
        for i in range(nb):
            if apply_gb:
                nrm = scr.tile([128, 128], F32, tag=tag + "_nrm")
                nc.vector.tensor_scalar(nrm[:], src[:, i, :], m[:, i:i + 1],
                                        rs[:, i:i + 1], OP.subtract, OP.mult)
                nrm2 = scr.tile([128, 128], F32, tag=tag + "_nrm2")
                nc.vector.tensor_tensor(nrm2[:], nrm[:], gB[:], OP.mult)
                nc.vector.tensor_tensor(dst[:, i, :], nrm2[:], bB[:], OP.add)
            else:
                nc.vector.tensor_scalar(dst[:, i, :], src[:, i, :],
                                        m[:, i:i + 1], rs[:, i:i + 1],
                                        OP.subtract, OP.mult)

    def transpose_blk(dst, src_ap, dtype, tpool=None):
        pt = (tpool or ps).tile([128, 128], dtype, tag="ps", name="pt")
        nc.tensor.transpose(pt[:], src_ap, ident_b[:] if dtype == BF16 else ident_f[:])
        nc.vector.tensor_copy(dst, pt[:])

    # =================== P1: pre (own tokens, T-layout) ===================
    _mark(nc, "P1_pre")
    preT = pool.tile([128, TOK], BF16, tag="preT")
    for n in range(NCH):
        pp = ps.tile([128, 512], F32, tag="ps")
        for k in range(8):
            nc.tensor.matmul(pp[:], wpre_sb[:, k, :], xt_sb[:, k, n * 512:(n + 1) * 512],
                             start=(k == 0), stop=(k == 7))
        nc.scalar.activation(preT[:, n * 512:(n + 1) * 512], pp[:],
                             AF.Identity, bias=bpre_c)

    # =================== P2: adapt_in (own) + AllGather ===================
    _mark(nc, "P2_lnin")
    preN = pool.tile([128, 8, AD], BF16, tag="preN")
    for i in range(8):
        transpose_blk(preN[:, i, :], preT[:, i * 128:(i + 1) * 128], BF16)
    ainN = pool.tile([128, 8, AD], BF16, tag="ainN")
    layer_norm(preN, 8, ainN, "lnin")
    ainT = pool.tile([128, TOK], BF16, tag="ainT")
    for i in range(8):
        transpose_blk(ainT[:, i * 128:(i + 1) * 128], ainN[:, i, :], BF16)

    # =================== P5: expert branch (needs only preT) ===============
    # Emitted before P3; the elementwise LN math runs on the otherwise-idle
    # Pool engine so it never contends with P3's DVE/Act chains. Matmuls go
    # to their own psum pool (phw), released before P3's pools open.
    _mark(nc, "P5_expert")
    # expert-weight prep (placed after P1/P2 so its loads + tiny PE matmuls
    # never head-of-line block the critical-path PE stream)
    eg_sb = pool.tile([1, E * AD], F32, tag="eg_sb")
    nc.gpsimd.dma_start(eg_sb[:], P["eg_row"][:])
    egr_bf = pool.tile([1, E * AD], BF16, tag="egr_bf")
    nc.gpsimd.tensor_copy(egr_bf[:], eg_sb[:])
    egB = pool.tile([128, E, AD], BF16, tag="egB")
    for e in range(E):
        nc.gpsimd.partition_broadcast(egB[:, e, :], egr_bf[:, e * AD:(e + 1) * AD])
    eb_f32 = pool.tile([E, AD], F32, tag="eb_f32")
    nc.gpsimd.dma_start(eb_f32[:], P["eb"][:])
    eb_nat = pool.tile([E, AD], BF16, tag="eb_nat")
    nc.gpsimd.tensor_copy(eb_nat[:], eb_f32[:])
    ew2_sb = pool.tile([128, E, 8], F32, tag="ew2_sb")
    nc.gpsimd.dma_start(ew2_sb[:], P["ew_p2"][:])
    ewr2 = pool.tile([128, E, 8], F32, tag="ewr2")
    nc.vector.tensor_scalar_max(ewr2[:], ew2_sb[:], 0.0)
    ewt_sb = pool.tile([E, TOK], F32, tag="ewt_sb")
    nc.gpsimd.dma_start(ewt_sb[:], P["ewt"][:])
    ewrT_sb = pool.tile([E, TOK], BF16, tag="ewrT_sb")
    nc.vector.tensor_scalar_max(ewrT_sb[:], ewt_sb[:], 0.0)
    ones8 = pool.tile([E, 1], F32, tag="ones8")
    nc.gpsimd.memset(ones8[:], 1.0)
    sumw_row = pool.tile([1, TOK], F32, tag="sumw_row")
    for n in range(NCH):
        psw = ps.tile([1, 512], F32, tag="ps")
        nc.tensor.matmul(psw[:], ones8[:], ewt_sb[:, n * 512:(n + 1) * 512],
                         start=True, stop=True)
        nc.vector.tensor_copy(sumw_row[:, n * 512:(n + 1) * 512], psw[:])
    sumwB = pool.tile([128, TOK], F32, tag="sumwB")
    nc.gpsimd.partition_broadcast(sumwB[:], sumw_row[:])

    phw_pool = tc.alloc_tile_pool(name=pfx + "phw", bufs=2, space="PSUM")
    ph_sb = pool.tile([128, E, 8, AD], BF16, tag="ph_sb")
    for e in range(E):
        phs = [phw_pool.tile([128, 4, AD], F32, tag="ph", name=f"ph{e}_{hb}")
               for hb in range(2)]
        for i in range(8):
            nc.tensor.matmul(phs[i // 4][:, i % 4, :],
                             preT[:, i * 128:(i + 1) * 128],
                             at_sb[:, e, :], start=True, stop=True)
        for hb in range(2):
            nc.scalar.activation(ph_sb[:, e, hb * 4:(hb + 1) * 4, :],
                                 phs[hb][:], AF.Copy)
    phw_pool.release()
    # The per-expert sum/sumsq reductions are emitted one per P3 iteration
    # (side_q) so the DVE queue never head-of-line blocks P3's hT combines.
    red_all = scr.tile([128, E, 8], F32, tag="x_red", bufs=1)
    red2_all = scr.tile([128, E, 8], F32, tag="x_red2", bufs=1)
    side_q = []
    _sq_tiles = {}

    def _mk_red(e):
        def f():
            nc.vector.tensor_reduce(red_all[:, e, :], ph_sb[:, e, :, :],
                                    AX.X, OP.add)
            sq = scr.tile([128, 8, AD], BF16, tag="x_sq", bufs=1)
            nc.gpsimd.tensor_tensor(sq[:], ph_sb[:, e, :, :],
                                    ph_sb[:, e, :, :], OP.mult)
            _sq_tiles[e] = sq
        def g():
            nc.vector.tensor_reduce(red2_all[:, e, :], _sq_tiles.pop(e)[:],
                                    AX.X, OP.add)
        return f, g

    for e in range(E):
        f, g = _mk_red(e)
        side_q.append(f)
        side_q.append(g)

    # =================== P3: up/gate -> hT, wpost accum ===================
    _mark(nc, "P3_upgate")
    hT = pool.tile([128, 16, TOK], BF16, tag="hT")
    ppo_pool = tc.alloc_tile_pool(name=pfx + "ppo_pool", bufs=2, space="PSUM")
    ppo = [ppo_pool.tile([128, 512], F32, tag="ppo", name=f"ppo{n}") for n in range(NCH)]
    with tc.tile_pool(name=pfx + "pug", bufs=2, space="PSUM") as pug:
        for ht in range(16):
            wu_ht = wpool.tile([128, 8, 128], BF16, tag="wu_ht", bufs=3)
            wg_ht = wpool.tile([128, 8, 128], BF16, tag="wg_ht", bufs=3)
            nc.sync.dma_start(wu_ht[:], P["wu_t"].ap()[ht])
            nc.sync.dma_start(wg_ht[:], P["wg_t"].ap()[ht])
            for n in range(NCH):
                pu = pug.tile([128, 512], F32, tag="pu")
                pg = pug.tile([128, 512], F32, tag="pg")
                for k in range(8):
                    nc.tensor.matmul(pu[:], wu_ht[:, k, :],
                                     xt_sb[:, k, n * 512:(n + 1) * 512],
                                     start=(k == 0), stop=(k == 7))
                for k in range(8):
                    nc.tensor.matmul(pg[:], wg_ht[:, k, :],
                                     xt_sb[:, k, n * 512:(n + 1) * 512],
                                     start=(k == 0), stop=(k == 7))
                silg = scr.tile([128, 512], F32, tag="silg")
                nc.scalar.activation(silg[:], pg[:], AF.Silu,
                                     bias=bgt[:, ht:ht + 1])
                nc.vector.scalar_tensor_tensor(
                    hT[:, ht, n * 512:(n + 1) * 512], pu[:], but[:, ht:ht + 1],
                    silg[:], OP.add, OP.mult)
                # wpost matmul for the PREVIOUS ht (software pipeline)
                if ht > 0:
                    nc.tensor.matmul(ppo[n][:], wpost_sb[:, ht - 1, :],
                                     hT[:, ht - 1, n * 512:(n + 1) * 512],
                                     start=(ht == 1), stop=False)
        for n in range(NCH):
            nc.tensor.matmul(ppo[n][:], wpost_sb[:, 15, :],
                             hT[:, 15, n * 512:(n + 1) * 512],
                             start=False, stop=True)

    # collective-1 (adapt_in gather): consumed only by the tail's pad
    # matmuls, so the send is emitted after P3 — its DMAs never steal
    # bandwidth from the critical early wu/wg weight stream.
    cc_in1 = dram.tile([128, 8, AD], BF16, tag="cc_in1")
    cc_out1 = dram.tile([2, 128, 8, AD], BF16, tag="cc_out1")
    nc.sync.dma_start(cc_in1[:], ainN[:])
    if fake_cc:
        nc.sync.dma_start(cc_out1[0], cc_in1[:])
        nc.sync.dma_start(cc_out1[1], cc_in1[:])
    else:
        nc.gpsimd.collective_compute(
            "AllGather", OP.bypass,
            replica_groups=[[0, 1], [2, 3], [4, 5], [6, 7]],
            ins=[cc_in1[:].opt()], outs=[cc_out1[:].opt()])
    ainN_f = pool.tile([128, 16, AD], BF16, tag="ainN_f")
    nc.scalar.dma_start(ainN_f[:, 0:8, :], cc_out1[0])
    nc.scalar.dma_start(ainN_f[:, 8:16, :], cc_out1[1])

    # =================== P4a: adapt_out LN + AllGather ===================
    _mark(nc, "P4a_aout")
    postT = pool.tile([128, TOK], BF16, tag="postT")
    for n in range(NCH):
        nc.vector.tensor_scalar_add(postT[:, n * 512:(n + 1) * 512], ppo[n][:],
                                    bpost_c)
    ppo_pool.release()
    postN = pool.tile([128, 8, AD], BF16, tag="postN")
    for i in range(8):
        transpose_blk(postN[:, i, :], postT[:, i * 128:(i + 1) * 128], BF16)
    aoutN = pool.tile([128, 8, AD], BF16, tag="aoutN")
    layer_norm(postN, 8, aoutN, "lnout", apply_gb=False)

    cc_in2 = dram.tile([128, 8, AD], BF16, tag="cc_in2")
    cc_out2 = dram.tile([2, 128, 8, AD], BF16, tag="cc_out2")
    nc.sync.dma_start(cc_in2[:], aoutN[:])
    if fake_cc:
        nc.sync.dma_start(cc_out2[0], cc_in2[:])
        nc.sync.dma_start(cc_out2[1], cc_in2[:])
    else:
        nc.gpsimd.collective_compute(
            "AllGather", OP.bypass,
            replica_groups=[[0, 1], [2, 3], [4, 5], [6, 7]],
            ins=[cc_in2[:].opt()], outs=[cc_out2[:].opt()])

    # wc/wda are only consumed by the final combine; load them here so the
    # DMAs never contend with the critical wu/wg stream.
    wc_sb = pool.tile([128, D], BF16, tag="wc_sb")
    nc.gpsimd.dma_start(wc_sb[:], P["wc"][:])
    wda_sb = pool.tile([128, D], BF16, tag="wda_sb")
    nc.gpsimd.dma_start(wda_sb[:], P["wda"][:])

    # =================== interleaved tail =====================
    # Emission order IS per-engine execution order, so the down-projection
    # psh groups (PE-heavy), collective readback, aoutT transposes, and the
    # aw/adapt steps are hand-woven: one adapt t-step between psh groups.
    _mark(nc, "P8a_down")
    aoutN_f = pool.tile([128, 16, AD], BF16, tag="aoutN_f")
    nc.scalar.dma_start(aoutN_f[:, 0:8, :], cc_out2[0])
    nc.scalar.dma_start(aoutN_f[:, 8:16, :], cc_out2[1])

    tcomb_sb = pool.tile([128, 8, TOK], BF16, tag="tcomb_sb")
    aoutT = pool.tile([128, S], BF16, tag="aoutT")
    adT = pool.tile([128, TOK], BF16, tag="adT")
    hwT = pool.tile([128, TOK], BF16, tag="hwT")
    pad_pool = tc.alloc_tile_pool(name=pfx + "pad_pool", bufs=2, space="PSUM")
    paw_pool = tc.alloc_tile_pool(name=pfx + "paw_pool", bufs=2, space="PSUM")
    psh_pool = tc.alloc_tile_pool(name=pfx + "psh", bufs=2, space="PSUM")
    pad = [pad_pool.tile([128, 512], F32, tag="pad", name=f"pad{n}")
           for n in range(NCH)]
    aw_tiles = {}
    tstep = [0]

    def emit_tstep():
        t = tstep[0]
        if t >= 16:
            return
        tstep[0] += 1
        # aoutT chunk t: transpose + fused ln_g/ln_b scale
        pt = ps.tile([128, 128], BF16, tag="ps", name="pt")
        nc.tensor.transpose(pt[:], aoutN_f[:, t, :], ident_b[:])
        nc.vector.tensor_scalar(aoutT[:, t * 128:(t + 1) * 128], pt[:],
                                lng_c, lnb_c, OP.mult, OP.add)
        for n in range(NCH):
            paw = paw_pool.tile([128, 512], F32, tag="paw")
            nc.tensor.matmul(paw[:], aoutT[:, t * 128:(t + 1) * 128],
                             ainT[:, n * 512:(n + 1) * 512],
                             start=True, stop=True)
            cl = scr.tile([128, 512], F32, tag="cl", bufs=2)
            nc.vector.tensor_scalar(cl[:], paw[:], 5.0, -5.0, OP.min, OP.max)
            aw_bf = scr.tile([128, 512], BF16, tag="aw_bf", bufs=3)
            nc.scalar.activation(aw_bf[:], cl[:], AF.Silu)
            aw_tiles[(t, n)] = aw_bf
            if t > 0:
                nc.tensor.matmul(pad[n][:], ainN_f[:, t - 1, :],
                                 aw_tiles.pop((t - 1, n))[:],
                                 start=(t == 1), stop=False)

    for s in range(16):
        dt, n = s // 2, s % 2
        c0, c1 = n * 512, (n + 1) * 512
        if n == 0:
            wd_dt = wpool.tile([128, 16, 128], BF16, tag="wd_dt")
            nc.sync.dma_start(wd_dt[:], P["wd_t"].ap()[dt])
        psh = psh_pool.tile([128, 512], F32, tag="psh")
        for k in range(16):
            nc.tensor.matmul(psh[:], wd_dt[:, k, :], hT[:, k, c0:c1],
                             start=(k == 0), stop=(k == 15))
        nc.vector.scalar_tensor_tensor(
            tcomb_sb[:, dt, c0:c1], psh[:], bdt[:, dt:dt + 1],
            sumwB[:, c0:c1], OP.add, OP.mult)
        if side_q:
            side_q.pop(0)()
        if side_q:
            side_q.pop(0)()
        if s >= 4:
            emit_tstep()
    psh_pool.release()

    # ---- expert LN: batched stats (Pool) + apply (Pool) -> hw ----
    # Emitted after P3's loop: the Act Sqrt sits behind all silgs in the
    # Act queue, and the Pool apply chain runs in the early tail window.
    hw = pool.tile([128, 8, AD], F32, tag="hw")
    t_all = scr.tile([128, E, 8], F32, tag="x_t")
    nc.gpsimd.tensor_tensor(t_all[:], red_all[:], red_all[:], OP.mult)
    v2_all = scr.tile([128, E, 8], F32, tag="x_v2")
    nc.vector.scalar_tensor_tensor(v2_all[:], t_all[:], -1.0 / AD, red2_all[:],
                                   OP.mult, OP.add)
    v3_all = scr.tile([128, E, 8], F32, tag="x_v3")
    nc.vector.tensor_scalar(v3_all[:], v2_all[:], 1.0 / AD, 1e-5,
                            OP.mult, OP.add)
    sd_all = scr.tile([128, E, 8], F32, tag="x_sd")
    nc.scalar.activation(sd_all[:], v3_all[:], AF.Sqrt)
    rs_all = scr.tile([128, E, 8], F32, tag="x_rs")
    nc.vector.reciprocal(rs_all[:], sd_all[:])
    rsw_all = scr.tile([128, E, 8], F32, tag="x_rsw", bufs=1)
    nc.gpsimd.tensor_tensor(rsw_all[:], rs_all[:], ewr2[:], OP.mult)
    nmrsw_all = scr.tile([128, E, 8], F32, tag="x_nmrsw", bufs=1)
    nc.vector.scalar_tensor_tensor(nmrsw_all[:], red_all[:], -1.0 / AD,
                                   rsw_all[:], OP.mult, OP.mult)
    for e in range(E):
        rswb = rsw_all[:, e, :].unsqueeze(2).broadcast_to([128, 8, AD])
        nmb = nmrsw_all[:, e, :].unsqueeze(2).broadcast_to([128, 8, AD])
        egv = egB[:, e, :].unsqueeze(1).broadcast_to([128, 8, AD])
        t2a = scr.tile([128, 8, AD], F32, tag="x_t2a", bufs=1)
        nc.gpsimd.tensor_tensor(t2a[:], ph_sb[:, e, :, :], rswb, OP.mult)
        nc.gpsimd.tensor_tensor(t2a[:], t2a[:], nmb, OP.add)
        if e == 0:
            nc.gpsimd.tensor_tensor(hw[:], t2a[:], egv, OP.mult)
        else:
            nc.gpsimd.tensor_tensor(t2a[:], t2a[:], egv, OP.mult)
            nc.gpsimd.tensor_tensor(hw[:], t2a[:], hw[:], OP.add)


    _mark(nc, "P6_tail")
    while tstep[0] < 16:
        emit_tstep()
    for n in range(NCH):
        nc.tensor.matmul(pad[n][:], ainN_f[:, 15, :],
                         aw_tiles.pop((15, n))[:], start=False, stop=True)
    # hwT[a, tok] = sum_e eb[e,a]*w_e[tok] + hw.T (eb rank-8 matmul + PE
    # transposes accumulated into the same psum)
    for half in range(2):
        pt = ps.tile([128, 512], F32, tag="ps", name=f"hwt{half}")
        nc.tensor.matmul(pt[:], eb_nat[:],
                         ewrT_sb[:, half * 512:(half + 1) * 512],
                         start=True, stop=False)
        for q in range(4):
            blk = half * 4 + q
            nc.tensor.matmul(pt[:, q * 128:(q + 1) * 128], hw[:, blk, :],
                             ident_f[:], is_transpose=True,
                             start=False, stop=(q == 3))
        nc.vector.tensor_copy(hwT[:, half * 512:(half + 1) * 512], pt[:])
    # adT = adapt.T * sumw (fold the sum(w) scaling of the adapt term here)
    for n in range(NCH):
        nc.vector.tensor_tensor(adT[:, n * 512:(n + 1) * 512], pad[n][:],
                                sumwB[:, n * 512:(n + 1) * 512], OP.mult)
    paw_pool.release()
    pad_pool.release()

    # =================== P8b: final combine + output ===================
    # out[dt, chunk] = tcomb + Wc.T@hwT + Wda.T@adT
    _mark(nc, "P8b_out")
    pct_pool = tc.alloc_tile_pool(name=pfx + "pct", bufs=4, space="PSUM")
    for s in range(16):
        dt, n = s % 8, s // 8
        c0, c1 = n * 512, (n + 1) * 512
        pct = pct_pool.tile([128, 512], F32, tag="pct")
        nc.tensor.matmul(pct[:], wc_sb[:, dt * 128:(dt + 1) * 128],
                         hwT[:, c0:c1], start=True, stop=False)
        nc.tensor.matmul(pct[:], wda_sb[:, dt * 128:(dt + 1) * 128],
                         adT[:, c0:c1], start=False, stop=True)
        osb = scr.tile([128, 512], BF16, tag="osb", name="osb", bufs=4)
        nc.vector.tensor_tensor(osb[:], tcomb_sb[:, dt, c0:c1], pct[:],
                                OP.add)
        nc.gpsimd.dma_start(P["out"].ap()[dt * 128:(dt + 1) * 128, c0:c1],
                            osb[:])
    pct_pool.release()

    stack.close()


def _prep_inputs(inputs):
    f = {k: np.asarray(v, np.float32) for k, v in inputs.items()}

    def swz(wt, nb):  # [K, M] -> [M/128, 128(p of K), K/128, 128] tiles
        k, mdim = wt.shape
        a = wt.reshape(k // 128, 128, nb, 128)
        return np.ascontiguousarray(a.transpose(2, 1, 0, 3)).astype(BF)

    def pk(w_t, nk, nf):  # [K, F] -> [128, nk, F]: (p,k,f) = w_t[k*128+p, f]
        return np.ascontiguousarray(
            w_t.reshape(nk, 128, nf).transpose(1, 0, 2)).astype(BF)

    wc = 0.1 * (f["Wo"] @ f["Wp"])      # [D, AD]
    wda = 0.1 * (f["Wd"] @ f["Wap"])    # [D, AD]

    shared = {
        "wu_t": swz(np.ascontiguousarray(f["Wu"].T), 16),
        "wg_t": swz(np.ascontiguousarray(f["Wg"].T), 16),
        "wd_t": swz(np.ascontiguousarray(f["Wd"].T), 8),
        "wpre_p": pk(np.ascontiguousarray(f["Wpre"].T), 8, AD),
        "wpost_p": pk(np.ascontiguousarray(f["Wpost"].T), 16, AD),
        "wc": np.ascontiguousarray(wc.T).astype(BF),
        "wda": np.ascontiguousarray(wda.T).astype(BF),
        "at_p": np.ascontiguousarray(f["A"].transpose(2, 0, 1)).astype(BF),
        "bias_pk": np.ascontiguousarray(np.concatenate([
            f["bu"].reshape(16, 128).T,
            f["bg"].reshape(16, 128).T,
            f["bd"].reshape(8, 128).T,
        ], axis=1)).astype(np.float32),
        "smalls": np.ascontiguousarray(np.stack(
            [f["bpre"], f["bpost"], f["ln_g"], f["ln_b"]], axis=1)),
        "smalls_row": np.concatenate([f["ln_g"], f["ln_b"]])[None, :].copy(),
        "eg_row": np.ascontiguousarray(f["eg"].reshape(1, E * AD)),
        "eb": f["eb"],
        "id_bf": np.eye(128, dtype=np.float32).astype(BF),
        "id_f32": np.eye(128, dtype=np.float32),
    }
    in_maps = []
    for c in range(N_CORES):
        b, j = c // 2, c % 2
        sl = slice(j * TOK, (j + 1) * TOK)
        m = dict(shared)
        xt = np.ascontiguousarray(f["x"][b, sl, :].T)  # [D, TOK]
        m["xt"] = np.ascontiguousarray(
            xt.reshape(8, 128, TOK).transpose(1, 0, 2)).astype(BF)
        m["ew_p2"] = np.ascontiguousarray(
            f["expert_weights"][b, sl, :].reshape(8, 128, E).transpose(1, 2, 0))
        m["ewt"] = np.ascontiguousarray(f["expert_weights"][b, sl, :].T)
        in_maps.append(m)
    return in_maps


def kernel(**inputs):
    global _NC_CACHE
    if _NC_CACHE is None:
        _NC_CACHE = build()
    in_maps = _prep_inputs(inputs)
    res = bass_utils.run_bass_kernel_spmd(
        _NC_CACHE, in_maps, core_ids=list(range(N_CORES)))
    out = np.empty((B, S, D), np.float32)
    for c in range(N_CORES):
        b, j = c // 2, c % 2
        out[b, j * TOK:(j + 1) * TOK, :] = res.results[c]["out"].T
    return out
